# revision 1
# baseline (speedup 1.0000x reference)
import math

import numpy as np

# GCNII layer constants (match the reference problem definition).
N = 100000
D = 32
ALPHA = 0.1
THETA = 0.5
LAYER = 8
BETA = math.log(THETA / (LAYER + 1) + 1.0)


def _compute_numpy(x, x_0, edge_index, weight1):
    src = np.asarray(edge_index[0], dtype=np.int64)
    dst = np.asarray(edge_index[1], dtype=np.int64)
    x = np.asarray(x, dtype=np.float32)
    x_0 = np.asarray(x_0, dtype=np.float32)
    weight1 = np.asarray(weight1, dtype=np.float32)

    # agg[i] = sum_{e: dst[e]==i} x[src[e]]  — one bincount per feature
    # column is much faster than np.add.at on a [E, D] gather.
    gathered = x[src]  # [E, D]
    agg = np.empty((N, D), dtype=np.float32)
    for d in range(D):
        agg[:, d] = np.bincount(dst, weights=gathered[:, d], minlength=N)

    out = (1.0 - ALPHA) * agg + ALPHA * x_0
    out = (1.0 - BETA) * out + BETA * (out @ weight1)
    return out.astype(np.float32)


def _compute_jax_neuron(x, x_0, edge_index, weight1):
    """Run the layer on the Trainium cores via PJRT, edges sharded 8-way.

    Each core takes a 1/8 slice of the edge list, gathers source rows from
    a replicated x, and scatter-adds into a full-size [N, D] partial;
    partials are summed across cores (psum), then the dense GCNII
    combination runs replicated.
    """
    import jax
    import jax.numpy as jnp
    from jax.sharding import Mesh, PartitionSpec as P
    from jax.experimental.shard_map import shard_map

    devs = jax.devices()
    n_cores = 8
    if len(devs) < n_cores:
        raise RuntimeError("need 8 cores")
    mesh = Mesh(np.array(devs[:n_cores]), ("i",))

    E = edge_index.shape[1]
    assert E % n_cores == 0

    src = jnp.asarray(np.asarray(edge_index[0], dtype=np.int32))
    dst = jnp.asarray(np.asarray(edge_index[1], dtype=np.int32))
    xj = jnp.asarray(x, dtype=jnp.float32)
    x0j = jnp.asarray(x_0, dtype=jnp.float32)
    wj = jnp.asarray(weight1, dtype=jnp.float32)

    def shard_fn(src_l, dst_l, x_full, x0_l, w):
        gathered = x_full[src_l]  # [E/8, D]
        partial = jax.ops.segment_sum(gathered, dst_l, num_segments=N)
        agg = jax.lax.psum(partial, "i")  # [N, D] replicated
        n_loc = x0_l.shape[0]
        idx = jax.lax.axis_index("i") * n_loc
        agg_l = jax.lax.dynamic_slice_in_dim(agg, idx, n_loc, axis=0)
        out = (1.0 - ALPHA) * agg_l + ALPHA * x0_l
        out = (1.0 - BETA) * out + BETA * (out @ w)
        return out

    fn = jax.jit(
        shard_map(
            shard_fn,
            mesh=mesh,
            in_specs=(P("i"), P("i"), P(), P("i"), P()),
            out_specs=P("i"),
        )
    )
    out = fn(src, dst, xj, x0j, wj)
    return np.asarray(jax.device_get(out), dtype=np.float32)


def kernel(x, x_0, edge_index, weight1):
    try:
        return _compute_jax_neuron(x, x_0, edge_index, weight1)
    except Exception:
        return _compute_numpy(x, x_0, edge_index, weight1)
